# revision 7
# baseline (speedup 1.0000x reference)
"""Multi-head attention (B=2, S=T=2048, D=1024, H=16) on 8 TRN2 NeuronCores.

Sharding: 2-way data parallel over batch x 4-way tensor parallel over heads.
Core c handles batch c//4 and heads [4*(c%4), 4*(c%4)+4).

Per-core kernel works entirely in transposed-activation layout:
  xT [k, t] (host pre-transposes), weights WT [k, d] (host pre-transposes),
  qT/kT [d_local, t], v [s, d_local], scoresT [s, t], cT [d_local, t],
  y [t, e] partial (host sums the 4 TP partials per batch).

Matmuls run in float32r (full-rate PE mode, ~1e-4 rel rounding); softmax
denominator is fused into the PV matmul as an extra all-ones stationary
column; normalization happens on the [65, t] PV output.
"""

import sys
import types

import numpy as np

import concourse.bass as bass  # noqa: F401  (registers engine classes)
import concourse.tile as tile
import concourse.mybir as mybir
from concourse import bacc
from concourse.bass import ts
from concourse.bass_utils import run_bass_kernel_spmd

FP32 = mybir.dt.float32
FP32R = mybir.dt.float32r
AF = mybir.ActivationFunctionType

D_MODEL = 1024
NUM_HEADS = 16
D_HEAD = 64
SCALING = D_HEAD ** -0.5
N_CORES = 8
DP = 2                      # data-parallel over batch
TPG = N_CORES // DP         # 4 tensor-parallel groups
DC = D_MODEL // TPG         # 256 output dims per core
HPC = DC // D_HEAD          # 4 heads per core
VCOL = D_HEAD + 1           # v columns per head incl. ones column

PROFILE = False             # set by test harness; collects exec_time_ns
LAST_EXEC_NS = None
LAST_RESULTS = None

_programs = {}


def _install_profile_hook():
    if "antenv.axon_hooks" in sys.modules:
        return
    try:
        from trn_agent_boot.trn_boot import _ntff_profile_via_ctypes
        hook = _ntff_profile_via_ctypes("/opt/axon/libaxon_pjrt.so")
    except Exception:
        hook = None
    mod = types.ModuleType("antenv.axon_hooks")
    mod.get_axon_ntff_profile_hook = lambda: hook
    mod.set_axon_ntff_profile_hook = lambda h: None
    sys.modules["antenv.axon_hooks"] = mod


def build_program(has_bias=False, has_mask=False, T=2048, S=2048, D=D_MODEL,
                  DCL=DC, TT=512):
    """Build the per-core bass program (SPMD: same program, per-core inputs)."""
    KC = D // 128            # contraction chunks
    KCH = KC // 2            # chunks per streamed half
    SC = S // 128            # s chunks (PV contraction)
    NTT = T // TT            # t tiles
    MC = DCL // 128          # qT/kT partition chunks
    HP = (DCL // D_HEAD) // 2  # head pairs
    NET = D // TT            # out-proj e tiles

    nc = bacc.Bacc("TRN2", target_bir_lowering=False, debug=False)
    xq_t = nc.dram_tensor("xq_t", [D, T], FP32R, kind="ExternalInput")
    xkv_t = nc.dram_tensor("xkv_t", [D, S], FP32R, kind="ExternalInput")
    wq_t = nc.dram_tensor("wq_t", [D, DCL], FP32R, kind="ExternalInput")
    wk_t = nc.dram_tensor("wk_t", [D, DCL], FP32R, kind="ExternalInput")
    wv_t = nc.dram_tensor("wv_t", [D, DCL], FP32R, kind="ExternalInput")
    wo_t = nc.dram_tensor("wo_t", [DCL, D], FP32R, kind="ExternalInput")
    y_t = nc.dram_tensor("y", [T, D], FP32, kind="ExternalOutput")
    if has_bias:
        bq_t = nc.dram_tensor("bq_t", [DCL], FP32, kind="ExternalInput")
        bk_t = nc.dram_tensor("bk_t", [DCL], FP32, kind="ExternalInput")
        bv_t = nc.dram_tensor("bv_t", [1, DCL], FP32R, kind="ExternalInput")
    if has_mask:
        mask_t = nc.dram_tensor("mask_t", [S, T], FP32, kind="ExternalInput")

    with tile.TileContext(nc) as tc:
        with tc.tile_pool(name="w", bufs=1) as wpool, \
             tc.tile_pool(name="big", bufs=1) as big, \
             tc.tile_pool(name="x", bufs=2) as xpool, \
             tc.tile_pool(name="e", bufs=3) as epool, \
             tc.tile_pool(name="r", bufs=2) as rpool, \
             tc.tile_pool(name="yst", bufs=2) as ypool:

            # ---- persistent weights / constants ----
            wq_sb = wpool.tile([128, KC, DCL], FP32R, tag="wq")
            nc.sync.dma_start(wq_sb[:], wq_t.ap().rearrange("(c p) d -> p c d", p=128))
            wk_sb = wpool.tile([128, KC, DCL], FP32R, tag="wk")
            nc.sync.dma_start(wk_sb[:], wk_t.ap().rearrange("(c p) d -> p c d", p=128))
            wv_sb = wpool.tile([128, KC, DCL], FP32R, tag="wv")
            nc.sync.dma_start(wv_sb[:], wv_t.ap().rearrange("(c p) d -> p c d", p=128))
            wo_sb = wpool.tile([128, MC, D], FP32R, tag="wo")
            nc.sync.dma_start(wo_sb[:], wo_t.ap().rearrange("(m p) e -> p m e", p=128))

            ones_row_f = wpool.tile([1, 128], FP32, tag="onesrowf")
            nc.gpsimd.memset(ones_row_f[:], 1.0)
            ones_row = wpool.tile([1, 128], FP32R, tag="onesrow")
            nc.vector.tensor_copy(ones_row[:], ones_row_f[:])
            ones_col_f = wpool.tile([128, 1], FP32, tag="onescolf")
            nc.gpsimd.memset(ones_col_f[:], 1.0)

            if has_bias:
                bq_sb = wpool.tile([128, MC], FP32, tag="bq")
                nc.sync.dma_start(bq_sb[:], bq_t.ap().rearrange("(m p) -> p m", p=128))
                bk_sb = wpool.tile([128, MC], FP32, tag="bk")
                nc.sync.dma_start(bk_sb[:], bk_t.ap().rearrange("(m p) -> p m", p=128))
                bv_sb = wpool.tile([1, DCL], FP32R, tag="bv")
                nc.sync.dma_start(bv_sb[:], bv_t.ap())

            # ---- persistent activations ----
            kT_sb = big.tile([128, MC, S], FP32R, tag="kT")
            qT_sb = big.tile([128, MC, T], FP32R, tag="qT")
            v_sb = big.tile([128, SC, HPC * VCOL], FP32R, tag="v")
            ct_sb = big.tile([128, MC, T], FP32R, tag="ct")

            # ones columns interleaved into v (denominator trick)
            for a in range(HPC):
                nc.vector.tensor_copy(
                    v_sb[:, :, a * VCOL + D_HEAD: (a + 1) * VCOL],
                    ones_col_f[:].to_broadcast((128, SC, 1)),
                )

            # ---- phase A: projections (contraction streamed in halves) ----
            with tc.tile_pool(name="psA", bufs=4, space="PSUM") as psA:
                for half in range(2):
                    xh = xpool.tile([128, KCH, S], FP32R, tag="xh")
                    for c in range(KCH):
                        nc.sync.dma_start(
                            xh[:, c, :],
                            xkv_t.ap()[(half * KCH + c) * 128:(half * KCH + c + 1) * 128, :])
                    # kT projection
                    for m in range(MC):
                        for st in range(S // TT):
                            ps = psA.tile([128, TT], FP32)
                            for c in range(KCH):
                                nc.tensor.matmul(
                                    ps[:], wk_sb[:, half * KCH + c, ts(m, 128)],
                                    xh[:, c, ts(st, TT)],
                                    start=(c == 0), stop=(c == KCH - 1))
                            dst = kT_sb[:, m, ts(st, TT)]
                            if half == 0:
                                if has_bias:
                                    nc.vector.tensor_scalar_add(dst, ps[:], bk_sb[:, m:m + 1])
                                else:
                                    nc.vector.tensor_copy(dst, ps[:])
                            else:
                                nc.vector.tensor_add(dst, dst, ps[:])
                    # v projection
                    for sc in range(SC):
                        ps = psA.tile([128, TT], FP32)
                        psv = ps[:, 0:DCL]
                        last_mm = (not has_bias) or half == 0
                        for c in range(KCH):
                            nc.tensor.matmul(
                                psv, xh[:, c, ts(sc, 128)], wv_sb[:, half * KCH + c, :],
                                start=(c == 0), stop=(c == KCH - 1 and last_mm))
                        if has_bias and half == 1:
                            nc.tensor.matmul(psv, ones_row[:], bv_sb[:],
                                             start=False, stop=True)
                        vdst = v_sb[:, sc, :].rearrange("p (h c) -> p h c", c=VCOL)[:, :, 0:D_HEAD]
                        psv_h = psv.rearrange("p (h c) -> p h c", c=D_HEAD)
                        if half == 0:
                            nc.vector.tensor_copy(vdst, psv_h)
                        else:
                            nc.vector.tensor_add(vdst, vdst, psv_h)
                for half in range(2):
                    xh = xpool.tile([128, KCH, T], FP32R, tag="xh")
                    for c in range(KCH):
                        nc.sync.dma_start(
                            xh[:, c, :],
                            xq_t.ap()[(half * KCH + c) * 128:(half * KCH + c + 1) * 128, :])
                    for m in range(MC):
                        for tt in range(NTT):
                            ps = psA.tile([128, TT], FP32)
                            for c in range(KCH):
                                nc.tensor.matmul(
                                    ps[:], wq_sb[:, half * KCH + c, ts(m, 128)],
                                    xh[:, c, ts(tt, TT)],
                                    start=(c == 0), stop=(c == KCH - 1))
                            dst = qT_sb[:, m, ts(tt, TT)]
                            if half == 0:
                                if has_bias:
                                    nc.vector.tensor_scalar_add(dst, ps[:], bq_sb[:, m:m + 1])
                                else:
                                    nc.vector.tensor_copy(dst, ps[:])
                            else:
                                nc.vector.tensor_add(dst, dst, ps[:])

            # ---- phase B: attention ----
            with tc.tile_pool(name="psS", bufs=2, space="PSUM") as psS, \
                 tc.tile_pool(name="psC", bufs=4, space="PSUM") as psC, \
                 (tc.tile_pool(name="msk", bufs=4) if has_mask else _nullpool()) as mpool:
                for hp in range(HP):
                    ha, hb = 2 * hp, 2 * hp + 1
                    for tt in range(NTT):
                        cA = psC.tile([128, TT], FP32, tag="c")
                        cB = psC.tile([128, TT], FP32, tag="c")
                        for sc in range(SC):
                            sAB = psS.tile([128, 2 * TT], FP32, tag="s")
                            nc.tensor.matmul(
                                sAB[:, 0:TT], kT_sb[0:64, hp, ts(sc, 128)],
                                qT_sb[0:64, hp, ts(tt, TT)], start=True, stop=True)
                            nc.tensor.matmul(
                                sAB[:, TT:2 * TT], kT_sb[64:128, hp, ts(sc, 128)],
                                qT_sb[64:128, hp, ts(tt, TT)], start=True, stop=True)
                            if has_mask:
                                mt = mpool.tile([128, TT], FP32, tag="m")
                                nc.sync.dma_start(
                                    mt[:], mask_t.ap()[ts(sc, 128), ts(tt, TT)])
                                nc.vector.tensor_add(sAB[:, 0:TT], sAB[:, 0:TT], mt[:])
                                nc.vector.tensor_add(sAB[:, TT:2 * TT], sAB[:, TT:2 * TT], mt[:])
                            eAB = epool.tile([128, 2 * TT], FP32R, tag="e")
                            nc.scalar.activation(eAB[:], sAB[:], AF.Exp)
                            nc.tensor.matmul(
                                cA[0:VCOL, :], v_sb[:, sc, ha * VCOL:(ha + 1) * VCOL],
                                eAB[:, 0:TT], start=(sc == 0), stop=(sc == SC - 1))
                            nc.tensor.matmul(
                                cB[0:VCOL, :], v_sb[:, sc, hb * VCOL:(hb + 1) * VCOL],
                                eAB[:, TT:2 * TT], start=(sc == 0), stop=(sc == SC - 1))
                        for cps, lh in ((cA, 0), (cB, 64)):
                            den = rpool.tile([1, TT], FP32, tag="den")
                            nc.vector.tensor_copy(den[:], cps[D_HEAD:VCOL, :])
                            rec_f = rpool.tile([1, TT], FP32, tag="recf")
                            nc.vector.reciprocal_approx_fast(rec_f[:], den[:])
                            rep = rpool.tile([64, TT], FP32, tag="rep")
                            nc.gpsimd.partition_broadcast(rep[:], rec_f[:])
                            dst = ct_sb[lh:lh + 64, hp, ts(tt, TT)]
                            nc.vector.tensor_mul(dst, cps[0:64, :], rep[:])

            # ---- phase C: output projection ----
            with tc.tile_pool(name="psO", bufs=4, space="PSUM") as psO:
                for tq in range(T // 128):
                    ysb = ypool.tile([128, D], FP32, tag="y")
                    for et in range(NET):
                        ps = psO.tile([128, TT], FP32)
                        for m in range(MC):
                            nc.tensor.matmul(
                                ps[:], ct_sb[:, m, ts(tq, 128)], wo_sb[:, m, ts(et, TT)],
                                start=(m == 0), stop=(m == MC - 1))
                        nc.vector.tensor_copy(ysb[:, ts(et, TT)], ps[:])
                    nc.sync.dma_start(y_t.ap()[ts(tq, 128), :], ysb[:])

    nc.compile()
    return nc


class _nullpool:
    def __enter__(self):
        return None

    def __exit__(self, *a):
        return False


def _get_program(has_bias, has_mask):
    key = (has_bias, has_mask)
    if key not in _programs:
        _programs[key] = build_program(has_bias, has_mask)
    return _programs[key]


def kernel(query_states, key_value_states, attention_mask,
           Wq, bq, Wk, bk, Wv, bv, Wo, bo):
    global LAST_EXEC_NS, LAST_RESULTS
    q = np.ascontiguousarray(np.asarray(query_states, dtype=np.float32))
    kv = np.ascontiguousarray(np.asarray(key_value_states, dtype=np.float32))
    mask = np.asarray(attention_mask, dtype=np.float32)
    Wq = np.asarray(Wq, np.float32); bq = np.asarray(bq, np.float32)
    Wk = np.asarray(Wk, np.float32); bk = np.asarray(bk, np.float32)
    Wv = np.asarray(Wv, np.float32); bv = np.asarray(bv, np.float32)
    Wo = np.asarray(Wo, np.float32); bo = np.asarray(bo, np.float32)

    has_bias = bool(np.any(bq) or np.any(bk) or np.any(bv))
    has_mask = bool(np.any(mask))
    nc = _get_program(has_bias, has_mask)

    in_maps = []
    for c in range(N_CORES):
        b, hg = divmod(c, TPG)
        sl = slice(DC * hg, DC * (hg + 1))
        m = {
            "xq_t": np.ascontiguousarray(q[b].T),
            "xkv_t": np.ascontiguousarray(kv[b].T),
            "wq_t": np.ascontiguousarray((Wq[sl] * SCALING).T),
            "wk_t": np.ascontiguousarray(Wk[sl].T),
            "wv_t": np.ascontiguousarray(Wv[sl].T),
            "wo_t": np.ascontiguousarray(Wo[:, sl].T),
        }
        if has_bias:
            m["bq_t"] = np.ascontiguousarray(bq[sl] * SCALING)
            m["bk_t"] = np.ascontiguousarray(bk[sl])
            m["bv_t"] = np.ascontiguousarray(bv[sl][None, :])
        if has_mask:
            mb = np.broadcast_to(mask[b].reshape(-1, mask.shape[-2], mask.shape[-1])[0],
                                 (q.shape[1], kv.shape[1]))
            m["mask_t"] = np.ascontiguousarray(mb.T)
        in_maps.append(m)

    if PROFILE:
        _install_profile_hook()
    res = run_bass_kernel_spmd(nc, in_maps, core_ids=list(range(N_CORES)),
                               trace=bool(PROFILE))
    LAST_EXEC_NS = res.exec_time_ns
    LAST_RESULTS = res
    outs = [res.results[c]["y"] for c in range(N_CORES)]
    y = np.stack([sum(outs[b * TPG:(b + 1) * TPG]) for b in range(DP)])
    return (y + bo).astype(np.float32)


# revision 12
# speedup vs baseline: 1.0201x; 1.0201x over previous
"""Multi-head attention (B=2, S=T=2048, D=1024, H=16) on 8 TRN2 NeuronCores.

Sharding: 2-way data parallel over batch x 4-way tensor parallel over heads.
Core c handles batch c//4 and heads [4*(c%4), 4*(c%4)+4).

Per-core kernel works entirely in transposed-activation layout:
  xT [k, t] (host pre-transposes), weights WT [k, d] (host pre-transposes),
  qT/kT [d_local, t], v [s, d_local], scoresT [s, t], cT [d_local, t],
  y [t, e] partial (host sums the 4 TP partials per batch).

Matmuls run in float32r (full-rate PE mode, ~1e-4 rel rounding); softmax
denominator is fused into the PV matmul as an extra all-ones stationary
column; normalization happens on the [65, t] PV output.
"""

import sys
import types

import numpy as np

import concourse.bass as bass  # noqa: F401  (registers engine classes)
import concourse.tile as tile
import concourse.mybir as mybir
from concourse import bacc
from concourse.bass import ts
from concourse.bass_utils import run_bass_kernel_spmd

FP32 = mybir.dt.float32
FP32R = mybir.dt.float32r
AF = mybir.ActivationFunctionType

D_MODEL = 1024
NUM_HEADS = 16
D_HEAD = 64
SCALING = D_HEAD ** -0.5
N_CORES = 8
DP = 2                      # data-parallel over batch
TPG = N_CORES // DP         # 4 tensor-parallel groups
DC = D_MODEL // TPG         # 256 output dims per core
HPC = DC // D_HEAD          # 4 heads per core
VCOL = D_HEAD + 1           # v columns per head incl. ones column

PROFILE = False             # set by test harness; collects exec_time_ns
LAST_EXEC_NS = None
LAST_RESULTS = None

_programs = {}


def _install_profile_hook():
    if "antenv.axon_hooks" in sys.modules:
        return
    try:
        from trn_agent_boot.trn_boot import _ntff_profile_via_ctypes
        hook = _ntff_profile_via_ctypes("/opt/axon/libaxon_pjrt.so")
    except Exception:
        hook = None
    mod = types.ModuleType("antenv.axon_hooks")
    mod.get_axon_ntff_profile_hook = lambda: hook
    mod.set_axon_ntff_profile_hook = lambda h: None
    sys.modules["antenv.axon_hooks"] = mod


def build_program(has_bias=False, has_mask=False, T=2048, S=2048, D=D_MODEL,
                  DCL=DC, TT=512):
    """Build the per-core bass program (SPMD: same program, per-core inputs)."""
    KC = D // 128            # contraction chunks
    KCH = KC // 2            # chunks per streamed half
    SC = S // 128            # s chunks (PV contraction)
    NTT = T // TT            # t tiles
    MC = DCL // 128          # qT/kT partition chunks
    HP = (DCL // D_HEAD) // 2  # head pairs
    NET = D // TT            # out-proj e tiles

    nc = bacc.Bacc("TRN2", target_bir_lowering=False, debug=False)
    xq_t = nc.dram_tensor("xq_t", [D, T], FP32R, kind="ExternalInput")
    xkv_t = nc.dram_tensor("xkv_t", [D, S], FP32R, kind="ExternalInput")
    wq_t = nc.dram_tensor("wq_t", [D, DCL], FP32R, kind="ExternalInput")
    wk_t = nc.dram_tensor("wk_t", [D, DCL], FP32R, kind="ExternalInput")
    wv_t = nc.dram_tensor("wv_t", [D, DCL], FP32R, kind="ExternalInput")
    wo_t = nc.dram_tensor("wo_t", [DCL, D], FP32R, kind="ExternalInput")
    y_t = nc.dram_tensor("y", [T, D], FP32, kind="ExternalOutput")
    if has_bias:
        bq_t = nc.dram_tensor("bq_t", [DCL], FP32, kind="ExternalInput")
        bk_t = nc.dram_tensor("bk_t", [DCL], FP32, kind="ExternalInput")
        bv_t = nc.dram_tensor("bv_t", [1, DCL], FP32R, kind="ExternalInput")
    if has_mask:
        mask_t = nc.dram_tensor("mask_t", [S, T], FP32, kind="ExternalInput")

    with tile.TileContext(nc) as tc:
        with tc.tile_pool(name="w", bufs=1) as wpool, \
             tc.tile_pool(name="big", bufs=1) as big, \
             tc.tile_pool(name="x", bufs=2) as xpool, \
             tc.tile_pool(name="e", bufs=3) as epool, \
             tc.tile_pool(name="r", bufs=2) as rpool, \
             tc.tile_pool(name="yst", bufs=2) as ypool:

            # ---- persistent weights / constants ----
            wk_sb = wpool.tile([128, KC, DCL], FP32R, tag="wk")
            nc.scalar.dma_start(wk_sb[:], wk_t.ap().rearrange("(c p) d -> p c d", p=128))
            wv_sb = wpool.tile([128, KC, DCL], FP32R, tag="wv")
            nc.scalar.dma_start(wv_sb[:], wv_t.ap().rearrange("(c p) d -> p c d", p=128))
            wq_sb = wpool.tile([128, KC, DCL], FP32R, tag="wq")
            nc.scalar.dma_start(wq_sb[:], wq_t.ap().rearrange("(c p) d -> p c d", p=128))
            wo_sb = wpool.tile([128, MC, D], FP32R, tag="wo")
            nc.scalar.dma_start(wo_sb[:], wo_t.ap().rearrange("(m p) e -> p m e", p=128))

            ones_row_f = wpool.tile([1, 128], FP32, tag="onesrowf")
            nc.gpsimd.memset(ones_row_f[:], 1.0)
            ones_row = wpool.tile([1, 128], FP32R, tag="onesrow")
            nc.vector.tensor_copy(ones_row[:], ones_row_f[:])
            ones_col_f = wpool.tile([128, 1], FP32, tag="onescolf")
            nc.gpsimd.memset(ones_col_f[:], 1.0)

            if has_bias:
                bq_sb = wpool.tile([128, MC], FP32, tag="bq")
                nc.sync.dma_start(bq_sb[:], bq_t.ap().rearrange("(m p) -> p m", p=128))
                bk_sb = wpool.tile([128, MC], FP32, tag="bk")
                nc.sync.dma_start(bk_sb[:], bk_t.ap().rearrange("(m p) -> p m", p=128))
                bv_sb = wpool.tile([1, DCL], FP32R, tag="bv")
                nc.sync.dma_start(bv_sb[:], bv_t.ap())

            # ---- persistent activations ----
            kT_sb = big.tile([128, MC, S], FP32R, tag="kT")
            qT_sb = big.tile([128, MC, T], FP32R, tag="qT")
            v_sb = big.tile([128, SC, HPC * VCOL], FP32R, tag="v")
            ct_sb = big.tile([128, MC, T], FP32R, tag="ct")

            # ones columns interleaved into v (denominator trick)
            for a in range(HPC):
                nc.vector.tensor_copy(
                    v_sb[:, :, a * VCOL + D_HEAD: (a + 1) * VCOL],
                    ones_col_f[:].to_broadcast((128, SC, 1)),
                )

            # ---- phase A: projections (contraction streamed in halves) ----
            with tc.tile_pool(name="psA", bufs=4, space="PSUM") as psA:
                for half in range(2):
                    xh = xpool.tile([128, KCH, S], FP32R, tag="xh")
                    for c in range(KCH):
                        nc.sync.dma_start(
                            xh[:, c, :],
                            xkv_t.ap()[(half * KCH + c) * 128:(half * KCH + c + 1) * 128, :])
                    # kT projection
                    for m in range(MC):
                        for st in range(S // TT):
                            ps = psA.tile([128, TT], FP32)
                            for c in range(KCH):
                                nc.tensor.matmul(
                                    ps[:], wk_sb[:, half * KCH + c, ts(m, 128)],
                                    xh[:, c, ts(st, TT)],
                                    start=(c == 0), stop=(c == KCH - 1))
                            dst = kT_sb[:, m, ts(st, TT)]
                            if half == 0:
                                if has_bias:
                                    nc.vector.tensor_scalar_add(dst, ps[:], bk_sb[:, m:m + 1])
                                else:
                                    nc.scalar.activation(dst, ps[:], AF.Copy)
                            else:
                                nc.vector.tensor_add(dst, dst, ps[:])
                    # v projection
                    for sc in range(SC):
                        ps = psA.tile([128, TT], FP32)
                        psv = ps[:, 0:DCL]
                        last_mm = (not has_bias) or half == 0
                        for c in range(KCH):
                            nc.tensor.matmul(
                                psv, xh[:, c, ts(sc, 128)], wv_sb[:, half * KCH + c, :],
                                start=(c == 0), stop=(c == KCH - 1 and last_mm))
                        if has_bias and half == 1:
                            nc.tensor.matmul(psv, ones_row[:], bv_sb[:],
                                             start=False, stop=True)
                        vdst = v_sb[:, sc, :].rearrange("p (h c) -> p h c", c=VCOL)[:, :, 0:D_HEAD]
                        psv_h = psv.rearrange("p (h c) -> p h c", c=D_HEAD)
                        if half == 0:
                            nc.vector.tensor_copy(vdst, psv_h)
                        else:
                            nc.vector.tensor_add(vdst, vdst, psv_h)
                for half in range(2):
                    xh = xpool.tile([128, KCH, T], FP32R, tag="xh")
                    for c in range(KCH):
                        nc.sync.dma_start(
                            xh[:, c, :],
                            xq_t.ap()[(half * KCH + c) * 128:(half * KCH + c + 1) * 128, :])
                    for m in range(MC):
                        for tt in range(NTT):
                            ps = psA.tile([128, TT], FP32)
                            for c in range(KCH):
                                nc.tensor.matmul(
                                    ps[:], wq_sb[:, half * KCH + c, ts(m, 128)],
                                    xh[:, c, ts(tt, TT)],
                                    start=(c == 0), stop=(c == KCH - 1))
                            dst = qT_sb[:, m, ts(tt, TT)]
                            if half == 0:
                                if has_bias:
                                    nc.vector.tensor_scalar_add(dst, ps[:], bq_sb[:, m:m + 1])
                                else:
                                    nc.scalar.activation(dst, ps[:], AF.Copy)
                            else:
                                nc.vector.tensor_add(dst, dst, ps[:])

            # ---- phase B: attention + folded output projection ----
            with tc.tile_pool(name="psS", bufs=2, space="PSUM") as psS, \
                 tc.tile_pool(name="psC", bufs=4, space="PSUM") as psC, \
                 (tc.tile_pool(name="msk", bufs=4) if has_mask else _nullpool()) as mpool:
                for tt in range(NTT):
                    for hp in range(HP):
                        ha, hb = 2 * hp, 2 * hp + 1
                        cA = psC.tile([128, TT], FP32, tag="c")
                        cB = psC.tile([128, TT], FP32, tag="c")
                        for sc in range(SC):
                            sAB = psS.tile([128, 2 * TT], FP32, tag="s")
                            nc.tensor.matmul(
                                sAB[:, 0:TT], kT_sb[0:64, hp, ts(sc, 128)],
                                qT_sb[0:64, hp, ts(tt, TT)], start=True, stop=True)
                            nc.tensor.matmul(
                                sAB[:, TT:2 * TT], kT_sb[64:128, hp, ts(sc, 128)],
                                qT_sb[64:128, hp, ts(tt, TT)], start=True, stop=True)
                            if has_mask:
                                mt = mpool.tile([128, TT], FP32, tag="m")
                                nc.sync.dma_start(
                                    mt[:], mask_t.ap()[ts(sc, 128), ts(tt, TT)])
                                nc.vector.tensor_add(sAB[:, 0:TT], sAB[:, 0:TT], mt[:])
                                nc.vector.tensor_add(sAB[:, TT:2 * TT], sAB[:, TT:2 * TT], mt[:])
                            eAB = epool.tile([128, 2 * TT], FP32R, tag="e")
                            nc.scalar.activation(eAB[:], sAB[:], AF.Exp)
                            nc.tensor.matmul(
                                cA[0:VCOL, :], v_sb[:, sc, ha * VCOL:(ha + 1) * VCOL],
                                eAB[:, 0:TT], start=(sc == 0), stop=(sc == SC - 1))
                            nc.tensor.matmul(
                                cB[0:VCOL, :], v_sb[:, sc, hb * VCOL:(hb + 1) * VCOL],
                                eAB[:, TT:2 * TT], start=(sc == 0), stop=(sc == SC - 1))
                        for cps, lh in ((cA, 0), (cB, 64)):
                            den = rpool.tile([1, TT], FP32, tag="den")
                            nc.vector.tensor_copy(den[:], cps[D_HEAD:VCOL, :])
                            rec_f = rpool.tile([1, TT], FP32, tag="recf")
                            nc.vector.reciprocal_approx_fast(rec_f[:], den[:])
                            rep = rpool.tile([64, TT], FP32, tag="rep")
                            nc.gpsimd.partition_broadcast(rep[:], rec_f[:])
                            dst = ct_sb[lh:lh + 64, hp, ts(tt, TT)]
                            nc.vector.tensor_mul(dst, cps[0:64, :], rep[:])

                    # output projection for this t-tile (all heads now done)
                    for tq in range(tt * (TT // 128), (tt + 1) * (TT // 128)):
                        ysb = ypool.tile([128, D], FP32, tag="y")
                        for et in range(NET):
                            ps = psC.tile([128, TT], FP32, tag="c")
                            for m in range(MC):
                                nc.tensor.matmul(
                                    ps[:], ct_sb[:, m, ts(tq, 128)], wo_sb[:, m, ts(et, TT)],
                                    start=(m == 0), stop=(m == MC - 1))
                            nc.vector.tensor_copy(ysb[:, ts(et, TT)], ps[:])
                        nc.sync.dma_start(y_t.ap()[ts(tq, 128), :], ysb[:])

    nc.compile()
    return nc


class _nullpool:
    def __enter__(self):
        return None

    def __exit__(self, *a):
        return False


def _get_program(has_bias, has_mask):
    key = (has_bias, has_mask)
    if key not in _programs:
        _programs[key] = build_program(has_bias, has_mask)
    return _programs[key]


def kernel(query_states, key_value_states, attention_mask,
           Wq, bq, Wk, bk, Wv, bv, Wo, bo):
    global LAST_EXEC_NS, LAST_RESULTS
    q = np.ascontiguousarray(np.asarray(query_states, dtype=np.float32))
    kv = np.ascontiguousarray(np.asarray(key_value_states, dtype=np.float32))
    mask = np.asarray(attention_mask, dtype=np.float32)
    Wq = np.asarray(Wq, np.float32); bq = np.asarray(bq, np.float32)
    Wk = np.asarray(Wk, np.float32); bk = np.asarray(bk, np.float32)
    Wv = np.asarray(Wv, np.float32); bv = np.asarray(bv, np.float32)
    Wo = np.asarray(Wo, np.float32); bo = np.asarray(bo, np.float32)

    has_bias = bool(np.any(bq) or np.any(bk) or np.any(bv))
    has_mask = bool(np.any(mask))
    nc = _get_program(has_bias, has_mask)

    in_maps = []
    for c in range(N_CORES):
        b, hg = divmod(c, TPG)
        sl = slice(DC * hg, DC * (hg + 1))
        m = {
            "xq_t": np.ascontiguousarray(q[b].T),
            "xkv_t": np.ascontiguousarray(kv[b].T),
            "wq_t": np.ascontiguousarray((Wq[sl] * SCALING).T),
            "wk_t": np.ascontiguousarray(Wk[sl].T),
            "wv_t": np.ascontiguousarray(Wv[sl].T),
            "wo_t": np.ascontiguousarray(Wo[:, sl].T),
        }
        if has_bias:
            m["bq_t"] = np.ascontiguousarray(bq[sl] * SCALING)
            m["bk_t"] = np.ascontiguousarray(bk[sl])
            m["bv_t"] = np.ascontiguousarray(bv[sl][None, :])
        if has_mask:
            mb = np.broadcast_to(mask[b].reshape(-1, mask.shape[-2], mask.shape[-1])[0],
                                 (q.shape[1], kv.shape[1]))
            m["mask_t"] = np.ascontiguousarray(mb.T)
        in_maps.append(m)

    if PROFILE:
        _install_profile_hook()
    res = run_bass_kernel_spmd(nc, in_maps, core_ids=list(range(N_CORES)),
                               trace=bool(PROFILE))
    LAST_EXEC_NS = res.exec_time_ns
    LAST_RESULTS = res
    outs = [res.results[c]["y"] for c in range(N_CORES)]
    y = np.stack([sum(outs[b * TPG:(b + 1) * TPG]) for b in range(DP)])
    return (y + bo).astype(np.float32)
